# revision 33
# baseline (speedup 1.0000x reference)
"""Self-contained Trainium2 Bass kernel for nn_ClusterForecasting.

Reference computation (B=4096, S=64, F=8, D=64, K=16):
    x_enc = x @ W_emb + b_emb                 [B,S,D]
    x_re  = x_enc.reshape(B, S*D)
    attn  = (x_re @ x_re.T) / sqrt(S*D), diag = -inf
    scores = softmax(attn, -1)                [B,B]
    x_rec = x_re * scores.sum(-1)             == x_re  (softmax rows sum to 1)
    y     = x_rec.reshape(B,S,D) @ W_proj + b_proj     [B,S,F]
    top_idx = top_k(scores, K)                (same indices as top_k of attn)
    gathered = y[top_idx]                     [B,K,S,F]
    loss = mean((y-x)^2) + mean(diff(gathered,1)^2) + mean(diff(gathered,2)^2)
    returns (loss, y)

Key reductions used here:
  * scores.sum(-1) == 1 exactly, so y = x @ (W_emb@W_proj) + (b_emb@W_proj + b_proj).
  * softmax is monotonic, so top_k indices == top_k of the unscaled Gram
    matrix G = x_re @ x_re.T (with the diagonal pushed to -BIG).
  * G = Xmat @ BD(M8) @ Xmat.T where Xmat = x.reshape(B, S*F),
    M8 = W_emb @ W_emb.T, and BD(.) tiles an [8,8] block diagonally per s.
    This contracts over S*F=512 instead of S*D=4096 (8x less PE work).
    NOTE: assumes b_emb == 0 for the Gram matrix (the problem spec pins
    b_emb to zeros); biases are still applied exactly in y.
  * The neighbor gather works on yT (y transposed, [S*F, B]) with the
    gpsimd ap_gather instruction (column gather): gathered columns are
    free-dim adjacent, so diff over k is a plain shifted subtraction.
  * diff over s needs partition shifts in the transposed layout, so it is
    instead folded per row: d2r[j] = sum((y[j,s+1,:]-y[j,s,:])^2) computed
    in the row layout, then ap_gather'd as scalars and summed.

Sharding: batch rows are sharded 512/core over 8 cores.  Every core runs the
identical SPMD program on a ROTATED copy of x (core i sees rows rolled by
512*i), so "rows 0..511" is always the local shard; no collectives are used.
The host stitches the 8 row shards of y and combines the per-core partial
sums into the loss.
"""

import numpy as np

import concourse.bass as bass
import concourse.tile as tile
from concourse import bacc, library_config, mybir
from concourse.bass_utils import run_bass_kernel_spmd

B, S, F, D, K = 4096, 64, 8, 64, 16
L = S * F            # 512, flattened (s,f) per sample
NCORES = 8
SHARD = B // NCORES  # 512 rows per core
NB = B // 128        # 32 b-chunks
RT = SHARD // 128    # 4 row-tiles per shard
DT = mybir.dt.float32
BIG = 1.0e30
SQ = mybir.ActivationFunctionType.Square

_CACHE = {}


def _build_program(variant="full"):
    nc = bacc.Bacc("TRN2", target_bir_lowering=False, debug=False)

    xin = nc.dram_tensor("xin", [B, L], DT, kind="ExternalInput")
    wemb = nc.dram_tensor("wemb", [F, D], DT, kind="ExternalInput")
    wproj = nc.dram_tensor("wproj", [D, F], DT, kind="ExternalInput")
    bemb = nc.dram_tensor("bemb", [D, 1], DT, kind="ExternalInput")
    bproj = nc.dram_tensor("bproj", [1, F], DT, kind="ExternalInput")
    ident = nc.dram_tensor("ident", [128, 128], DT, kind="ExternalInput")
    negbig = nc.dram_tensor("negbig", [128, 128], DT, kind="ExternalInput")

    out_y = nc.dram_tensor("out_y", [SHARD, L], DT, kind="ExternalOutput")
    out_partials = nc.dram_tensor("out_partials", [3, 1], DT, kind="ExternalOutput")

    with tile.TileContext(nc) as tc:
        with (
            tc.tile_pool(name="consts", bufs=1) as consts,
            tc.tile_pool(name="big", bufs=1) as bigpool,
            tc.tile_pool(name="smal", bufs=4) as smal,
            tc.tile_pool(name="pa", bufs=2, space="PSUM") as pap,   # pg/py tags
            tc.tile_pool(name="pb", bufs=4, space="PSUM") as pbp,   # shared tp tag
            tc.tile_pool(name="dram", bufs=1, space="DRAM") as dramp,
        ):
            # ---------------- phase 0: constants / small precompute ---------
            nc.gpsimd.load_library(library_config.ap_gather)

            ident_sb = consts.tile([128, 128], DT, tag="ident")
            nc.sync.dma_start(ident_sb[:], ident[:])
            negbig_sb = consts.tile([128, 128], DT, tag="negbig")
            nc.sync.dma_start(negbig_sb[:], negbig[:])
            wemb_sb = consts.tile([F, D], DT, tag="wemb")
            nc.sync.dma_start(wemb_sb[:], wemb[:])
            wproj_sb = consts.tile([D, F], DT, tag="wproj")
            nc.sync.dma_start(wproj_sb[:], wproj[:])
            bemb_sb = consts.tile([D, 1], DT, tag="bemb")
            nc.sync.dma_start(bemb_sb[:], bemb[:])
            bproj_sb = consts.tile([1, F], DT, tag="bproj")
            nc.sync.dma_start(bproj_sb[:], bproj[:])

            # W_emb.T via PE transpose: [8,64] -> [64,8]
            wembT_ps = pbp.tile([128, 128], DT, tag="tp")
            nc.tensor.transpose(wembT_ps[0:D, 0:F], wemb_sb[:], ident_sb[0:F, 0:F])
            wembT_sb = consts.tile([D, F], DT, tag="wembT")
            nc.scalar.copy(wembT_sb[:], wembT_ps[0:D, 0:F])

            # M8 = W_emb @ W_emb.T  and  Wc = W_emb @ W_proj   (both [8,8])
            m8_ps = pbp.tile([128, 128], DT, tag="tp")
            nc.tensor.matmul(m8_ps[0:F, 0:F], wembT_sb[:], wembT_sb[:],
                             start=True, stop=True)
            m8_sb = consts.tile([F, F], DT, tag="m8")
            nc.scalar.copy(m8_sb[:], m8_ps[0:F, 0:F])

            wc_ps = pbp.tile([128, 128], DT, tag="tp")
            nc.tensor.matmul(wc_ps[0:F, 0:F], wembT_sb[:], wproj_sb[:],
                             start=True, stop=True)
            wc_sb = consts.tile([F, F], DT, tag="wc")
            nc.scalar.copy(wc_sb[:], wc_ps[0:F, 0:F])

            # bc = b_emb @ W_proj + b_proj   [1,8], tiled to [1,512]
            bc_ps = pbp.tile([128, 128], DT, tag="tp")
            nc.tensor.matmul(bc_ps[0:1, 0:F], bemb_sb[:], wproj_sb[:],
                             start=True, stop=True)
            bc_sb = consts.tile([1, F], DT, tag="bc")
            nc.vector.tensor_add(bc_sb[:], bc_ps[0:1, 0:F], bproj_sb[:])
            bc512 = consts.tile([1, L], DT, tag="bc512")
            for s in range(S):
                nc.scalar.copy(bc512[:, s * F:(s + 1) * F], bc_sb[:])
            # bias as a per-partition column for the yT layout
            bccol_ps = pbp.tile([128, 128], DT, tag="tp")
            nc.tensor.transpose(bccol_ps[0:128, 0:1], bc512[:, 0:128],
                                ident_sb[0:1, 0:1])
            bc_col = consts.tile([128, 1], DT, tag="bccol")
            nc.scalar.copy(bc_col[:], bccol_ps[0:128, 0:1])

            # block-diagonal [128,128] tiles: 16 copies of M8 / Wc on the diag.
            # Compute engines can't start at partition 8j, so route via DRAM
            # and place the diagonal blocks with DMA (any partition allowed).
            m8_dram = dramp.tile([F, F], DT, tag="m8_dram")
            nc.sync.dma_start(m8_dram[:], m8_sb[:])
            wc_dram = dramp.tile([F, F], DT, tag="wc_dram")
            nc.sync.dma_start(wc_dram[:], wc_sb[:])
            bd_m8 = consts.tile([128, 128], DT, tag="bdm8")
            nc.vector.memset(bd_m8[:], 0.0)
            bd_wc = consts.tile([128, 128], DT, tag="bdwc")
            nc.vector.memset(bd_wc[:], 0.0)
            for j in range(16):
                nc.sync.dma_start(bd_m8[j * F:(j + 1) * F, j * F:(j + 1) * F],
                                  m8_dram[:])
                nc.sync.dma_start(bd_wc[j * F:(j + 1) * F, j * F:(j + 1) * F],
                                  wc_dram[:])

            ones_row = consts.tile([1, 128], DT, tag="ones_row")
            nc.vector.memset(ones_row[:], 1.0)
            ones_col = consts.tile([128, 1], DT, tag="ones_col")
            nc.vector.memset(ones_col[:], 1.0)
            # bias replicated across partitions (once) for the y-row layout
            bcf_ps = pbp.tile([128, 512], DT, tag="tp")
            nc.tensor.matmul(bcf_ps[:], ones_row[:], bc512[:], start=True, stop=True)
            bc512f = consts.tile([128, L], DT, tag="bc512f")
            nc.scalar.copy(bc512f[:], bcf_ps[:])
            # acc columns: 0=(y-x)^2, 1=diff1, 2=diff2(row 0 only)
            acc = consts.tile([128, 3], DT, tag="acc")
            nc.vector.memset(acc[:], 0.0)
            idxs_sb = consts.tile([128, 128], mybir.dt.int16, tag="idxs")
            scal_d2r = consts.tile([16, B], DT, tag="scald2r")
            nc.vector.memset(scal_d2r[:], 0.0)
            d2r_sb = consts.tile([128, NB], DT, tag="d2rsb")

            # big persistent tensors: xT / yT hold chunk k at cols [B*k, B*(k+1))
            xT = bigpool.tile([128, 4 * B], DT, tag="xT")
            yT = bigpool.tile([128, 4 * B], DT, tag="yT")
            zts = bigpool.tile([128, 4 * SHARD], DT, tag="zts")

            idx_dram = dramp.tile([128, 16], mybir.dt.uint16, tag="idx_dram")
            d2r_dram = dramp.tile([B], DT, tag="d2r_dram")

            # ---------------- phase 1: transpose x, compute y / yT ----------
            xio = tc.alloc_tile_pool(name="xio", bufs=2)
            ysbp = tc.alloc_tile_pool(name="ysb", bufs=2)
            for c in range(NB):
                x_sb = xio.tile([128, L], DT, tag="x")
                nc.sync.dma_start(x_sb[:], xin[c * 128:(c + 1) * 128, :])

                # 4 PE transposes: xT[:, B*k + 128c ..] = x_sb[:, 128k..].T
                for k in range(4):
                    t_ps = pbp.tile([128, 128], DT, tag="tp")
                    nc.tensor.transpose(t_ps[:], x_sb[:, k * 128:(k + 1) * 128],
                                        ident_sb[:])
                    dst = xT[:, B * k + 128 * c: B * k + 128 * (c + 1)]
                    if k % 2 == 0:
                        nc.scalar.copy(dst, t_ps[:])
                    else:
                        nc.vector.tensor_copy(dst, t_ps[:])

                # shard columns: ZT = BD(M8) @ xT   (only needed for b < 512)
                if c < RT:
                    for k in range(4):
                        z_ps = pbp.tile([128, 128], DT, tag="tp")
                        nc.tensor.matmul(
                            z_ps[:], bd_m8[:],
                            xT[:, B * k + 128 * c: B * k + 128 * (c + 1)],
                            start=True, stop=True)
                        nc.vector.tensor_copy(
                            zts[:, 512 * k + 128 * c: 512 * k + 128 * (c + 1)], z_ps[:])

                # yT = BD(Wc) @ xT + bc (bias per partition via Identity-copy)
                for k in range(4):
                    yt_ps = pbp.tile([128, 128], DT, tag="tp")
                    nc.tensor.matmul(
                        yt_ps[:], bd_wc[:],
                        xT[:, B * k + 128 * c: B * k + 128 * (c + 1)],
                        start=True, stop=True)
                    nc.scalar.activation(
                        yT[:, B * k + 128 * c: B * k + 128 * (c + 1)], yt_ps[:],
                        mybir.ActivationFunctionType.Identity, bias=bc_col[:])

                # y rows [128, 512]: band k = xT_k.T @ BD(Wc); bias added on DVE
                y_ps = pap.tile([128, L], DT, tag="py")
                for k in range(4):
                    # start=True zeroes the whole PSUM bank: only k==0 sets it;
                    # later bands hit pending-zero regions and write fresh.
                    nc.tensor.matmul(
                        y_ps[:, k * 128:(k + 1) * 128],
                        xT[:, B * k + 128 * c: B * k + 128 * (c + 1)],
                        bd_wc[:], start=(k == 0), stop=(k == 3),
                        skip_group_check=True)

                y_sb = ysbp.tile([128, L], DT, tag="y")
                nc.vector.tensor_add(y_sb[:], y_ps[:], bc512f[:])
                # d2r[row] = sum_s,f (y[s+1,f]-y[s,f])^2  (free-dim shift)
                ds2 = pbp.tile([128, 512], DT, tag="tp")
                nc.vector.tensor_sub(ds2[:, 0:L - F], y_sb[:, F:L], y_sb[:, 0:L - F])
                nc.scalar.activation(ds2[:, 0:L - F], ds2[:, 0:L - F], SQ,
                                     accum_out=d2r_sb[:, c:c + 1])
                if c < RT:
                    nc.sync.dma_start(out_y[c * 128:(c + 1) * 128, :], y_sb[:])
                    # partial sum of (y - x)^2 over the shard
                    ds = pbp.tile([128, 512], DT, tag="tp")
                    nc.vector.tensor_sub(ds[:], y_sb[:], x_sb[:])
                    tmp = smal.tile([128, 1], DT, tag="tmp")
                    nc.scalar.activation(ds[:], ds[:], SQ, accum_out=tmp[:])
                    nc.vector.tensor_add(acc[:, 0:1], acc[:, 0:1], tmp[:])

            # d2r -> dram (row-major j = p*NB + c) -> scal_d2r[0, b] (b=128c+p)
            nc.sync.dma_start(d2r_dram[:], d2r_sb[:])
            nc.sync.dma_start(
                scal_d2r[0:1, :],
                d2r_dram[:].rearrange("(p c) -> c p", c=NB))

            # phase-1-only pools released so phase-2 pools can reuse the space
            # (LIFO: pools must be released in reverse allocation order)
            ysbp.release()
            xio.release()
            scp = tc.alloc_tile_pool(name="sc", bufs=1)
            gatp = tc.alloc_tile_pool(name="gat", bufs=2)

            # ---------------- phase 2: G rows, topk, gather, diffs -----------
            for m in range(RT if variant != "y_only" else 0):
                scores = scp.tile([128, B], DT, tag="scores")
                for n in range(8):
                    g_ps = pap.tile([128, 512], DT, tag="pg")
                    for k in range(4):
                        nc.tensor.matmul(
                            g_ps[:],
                            zts[:, 512 * k + 128 * m: 512 * k + 128 * (m + 1)],
                            xT[:, B * k + 512 * n: B * k + 512 * (n + 1)],
                            start=(k == 0), stop=(k == 3 and n != 0),
                            skip_group_check=True)
                    if n == 0:
                        # mask the self-similarity diagonal with -BIG
                        nc.tensor.matmul(g_ps[:, 128 * m: 128 * (m + 1)],
                                         ident_sb[:], negbig_sb[:],
                                         start=False, stop=True,
                                         skip_group_check=True)
                    if n % 2 == 0:
                        nc.scalar.copy(scores[:, 512 * n: 512 * (n + 1)], g_ps[:])
                    else:
                        nc.vector.tensor_copy(scores[:, 512 * n: 512 * (n + 1)],
                                              g_ps[:])

                # top-16 per row: two rounds of max8
                topidx = smal.tile([128, 16], mybir.dt.uint16, tag="topidx")
                r1v = smal.tile([128, 8], DT, tag="r1v")
                nc.vector.max(r1v[:], scores[:])
                nc.vector.max_index(topidx[:, 0:8], r1v[:], scores[:])
                nc.vector.match_replace(scores[:], r1v[:], scores[:], -BIG)
                r2v = smal.tile([128, 8], DT, tag="r2v")
                nc.vector.max(r2v[:], scores[:])
                nc.vector.max_index(topidx[:, 8:16], r2v[:], scores[:])

                if variant == "no_gather":
                    continue
                # idxs for ap_gather: per 16-partition group g, wrap slot
                # [j%16, j//16] must hold idx of gathered column j = 16*b + k,
                # i.e. group block[k, b] = topidx[b, k] — transpose via DRAM.
                nc.sync.dma_start(idx_dram[:], topidx[:])
                for g in range(8):
                    nc.sync.dma_start(
                        idxs_sb[16 * g:16 * (g + 1), :],
                        idx_dram[:].rearrange("b k -> k b").bitcast(mybir.dt.int16))

                # diff2 partial: gather d2r scalars for all 2048 neighbors
                sc16 = gatp.tile([16, 2048], DT, tag="gat")
                nc.gpsimd.ap_gather(sc16[:], scal_d2r[:], idxs_sb[0:16, :],
                                    channels=16, num_elems=B, d=1, num_idxs=2048)
                tmp16 = smal.tile([16, 1], DT, tag="tmp16")
                nc.vector.tensor_reduce(tmp16[:], sc16[:],
                                        axis=mybir.AxisListType.X,
                                        op=mybir.AluOpType.add)
                nc.vector.tensor_add(acc[0:1, 2:3], acc[0:1, 2:3], tmp16[0:1, :])

                # diff1 partial: gather yT columns, then (col[i+1]-col[i])^2
                # over k-adjacent pairs (i = 16*b + k)
                for t in range(4):
                    gat = gatp.tile([128, 2048], DT, tag="gat")
                    nc.gpsimd.ap_gather(gat[:], yT[:, B * t: B * (t + 1)],
                                        idxs_sb[:], channels=128,
                                        num_elems=B, d=1, num_idxs=2048)
                    gv = gat[:].rearrange("p (b k) -> p b k", k=16)
                    for q in range(4):
                        d_ps = pbp.tile([128, 512], DT, tag="tp")
                        dv = d_ps[:, 0:480].rearrange("p (b k) -> p b k", k=15)
                        nc.vector.tensor_sub(dv, gv[:, 32 * q:32 * (q + 1), 1:16],
                                             gv[:, 32 * q:32 * (q + 1), 0:15])
                        tmp = smal.tile([128, 1], DT, tag="tmp")
                        nc.scalar.activation(d_ps[:, 0:480], d_ps[:, 0:480], SQ,
                                             accum_out=tmp[:])
                        nc.vector.tensor_add(acc[:, 1:2], acc[:, 1:2], tmp[:])

            # ---------------- phase 3: reduce partials across partitions -----
            fin_ps = pbp.tile([128, 128], DT, tag="tp")
            nc.tensor.matmul(fin_ps[0:3, 0:1], acc[:], ones_col[:],
                             start=True, stop=True)
            fin_sb = smal.tile([3, 1], DT, tag="fin")
            nc.scalar.copy(fin_sb[:], fin_ps[0:3, 0:1])
            nc.sync.dma_start(out_partials[:], fin_sb[:])
            gatp.release()
            scp.release()

    nc.compile()
    return nc


def _get_program():
    if "nc" not in _CACHE:
        _CACHE["nc"] = _build_program()
    return _CACHE["nc"]


def kernel(x, W_emb, b_emb, W_proj, b_proj):
    x = np.ascontiguousarray(np.asarray(x, dtype=np.float32))
    W_emb = np.ascontiguousarray(np.asarray(W_emb, dtype=np.float32))
    b_emb = np.ascontiguousarray(np.asarray(b_emb, dtype=np.float32))
    W_proj = np.ascontiguousarray(np.asarray(W_proj, dtype=np.float32))
    b_proj = np.ascontiguousarray(np.asarray(b_proj, dtype=np.float32))

    nc = _get_program()

    xflat = x.reshape(B, L)
    ident = np.eye(128, dtype=np.float32)
    negbig = (-BIG) * ident
    common = {
        "wemb": W_emb,
        "wproj": W_proj,
        "bemb": b_emb.reshape(D, 1),
        "bproj": b_proj.reshape(1, F),
        "ident": ident,
        "negbig": negbig,
    }
    in_maps = []
    for i in range(NCORES):
        r = SHARD * i
        xin = np.concatenate([xflat[r:], xflat[:r]], axis=0)
        in_maps.append({"xin": np.ascontiguousarray(xin), **common})

    res = run_bass_kernel_spmd(nc, in_maps, core_ids=list(range(NCORES)))
    _CACHE["last_result"] = res
    outs = res.results

    y_full = np.empty((B, L), dtype=np.float32)
    psum = np.zeros(3, dtype=np.float64)
    for i in range(NCORES):
        y_full[SHARD * i: SHARD * (i + 1)] = outs[i]["out_y"]
        psum += outs[i]["out_partials"].reshape(3).astype(np.float64)

    loss = (psum[0] / (B * S * F)
            + psum[1] / (B * (K - 1) * S * F)
            + psum[2] / (B * K * (S - 1) * F))
    return np.float32(loss), y_full.reshape(B, S, F)
